# revision 11
# baseline (speedup 1.0000x reference)
"""ADMM-Net (12 unrolled stages) on 8 TRN2 NeuronCores.

Data-parallel over the batch: each core owns a [N, B/8] column block of the
transposed problem. Per stage s (with state u, r where r = yT + rho*(z-u)):

    psum = W_s.T @ r            W_s = (diag(M3_s) M2_s).T, pre-folded on host
    t    = psum + u             (alpha == 1 fast path)
    u'   = clip(t, +-lam/rho)   == t - softthresh(t) ; becomes next u
    r'   = yT + rho*(t - 2u')   next stage matmul rhs
    z    = t - u'               (emitted only at the last stage)

Matmuls run in float32r (FP22 truncated reads, fp32 accumulate): full
1-cycle/row PE speed at 512-wide moving tiles, ~7e-4 final rel err.
All state stays fp32 in SBUF for the whole 12-stage chain; only the
per-stage weights stream from HBM.
"""

import os
import numpy as np

S, B, N = 12, 8192, 1024
NCORES = 8
BC = B // NCORES          # batch columns per core
P = 128                   # partitions
KT = N // P               # 8 contraction tiles
MT = N // P               # 8 output-row tiles
CHUNK = 512               # psum bank width (f32)
NB = BC // CHUNK          # 2 column chunks per core
KPACK = 1                 # k-slabs packed per weight DMA
E_OFFLOAD = "gpsimd"          # engine for the e = rho*t + Y op

_compiled: dict = {}
LAST_RESULT = None        # BassKernelResults of the most recent run (for test.py)


def _build(lam: tuple, rho: float):
    import concourse.tile as tile
    from concourse import bacc, mybir

    f32 = mybir.dt.float32
    f32r = mybir.dt.float32r
    Alu = mybir.AluOpType
    Act = mybir.ActivationFunctionType

    nc = bacc.Bacc("TRN2", target_bir_lowering=False, debug=False, num_devices=NCORES)
    yt_d = nc.dram_tensor("yt", [N, BC], f32r, kind="ExternalInput").ap()
    w_d = nc.dram_tensor("w", [S, KT // KPACK, P, KPACK * N], f32r, kind="ExternalInput").ap()
    out_d = nc.dram_tensor("out", [N, BC], f32, kind="ExternalOutput").ap()

    with tile.TileContext(nc) as tc:
        with (
            tc.tile_pool(name="state", bufs=1) as state,
            tc.tile_pool(name="wpool", bufs=13) as wpool,
            tc.tile_pool(name="scratch", bufs=3) as scratch,
            tc.tile_pool(name="fin", bufs=1) as fin,
            tc.tile_pool(name="psum", bufs=6, space="PSUM") as psum_pool,
        ):
            Yt = [state.tile([P, BC], f32r, tag=f"Y{i}", name=f"Y{i}") for i in range(KT)]
            for i in range(KT):
                nc.sync.dma_start(Yt[i][:, 0:CHUNK], yt_d[i * P : (i + 1) * P, 0:CHUNK])
            ut = [state.tile([P, BC], f32, tag=f"u{i}", name=f"u{i}") for i in range(MT)]
            rt = [
                [state.tile([P, BC], f32r, tag=f"r{p}_{i}", name=f"r{p}_{i}") for i in range(KT)]
                for p in range(2)
            ]

            for s in range(S):
                lam_s = float(lam[s])
                rhs = Yt if s == 0 else rt[(s - 1) % 2]
                rnew = rt[s % 2]

                slabs = [wpool.tile([P, KPACK * N], f32r, tag="w", name=f"w{s}_{i}") for i in range(KT // KPACK)]
                for kk in range(KT // KPACK):
                    nc.sync.dma_start(slabs[kk][:], w_d[s, kk, :, :])
                if s == 0:
                    # yT b1-halves: needed only mid-stage-0; issue after stage-0 weights
                    for i in range(KT):
                        nc.sync.dma_start(
                            Yt[i][:, CHUNK:BC], yt_d[i * P : (i + 1) * P, CHUNK:BC]
                        )

                def wtile(k, m):
                    sl = slabs[k // KPACK]
                    off = (k % KPACK) * N + m * P
                    return sl[:, off : off + P]

                for b in range(NB):
                    bs = slice(b * CHUNK, (b + 1) * CHUNK)
                    for m in range(MT):
                        ps = psum_pool.tile([P, CHUNK], f32, tag="ps", name=f"ps{s}_{m}_{b}")
                        for k in range(KT):
                            nc.tensor.matmul(
                                ps[:],
                                wtile(k, m),
                                rhs[k][:, bs],
                                start=(k == 0),
                                stop=(k == KT - 1),
                            )
                        t_t = scratch.tile([P, CHUNK], f32, tag="t", name=f"t{s}_{m}_{b}")
                        if s == 0:
                            # u == 0: t = psum
                            nc.scalar.activation(t_t[:], ps[:], Act.Copy)
                        else:
                            nc.vector.tensor_tensor(t_t[:], ps[:], ut[m][:, bs], Alu.add)
                        if s == S - 1:
                            uc = fin.tile([P, CHUNK], f32, tag="uc", name=f"uc{m}_{b}")
                            nc.gpsimd.tensor_scalar(uc[:], t_t[:], lam_s, -lam_s, Alu.min, Alu.max)
                            z_t = fin.tile([P, CHUNK], f32, tag="z", name=f"z{m}_{b}")
                            nc.vector.tensor_tensor(z_t[:], t_t[:], uc[:], Alu.subtract)
                            nc.sync.dma_start(out_d[m * P : (m + 1) * P, bs], z_t[:])
                        else:
                            nc.gpsimd.tensor_scalar(ut[m][:, bs], t_t[:], lam_s, -lam_s, Alu.min, Alu.max)
                            e_t = scratch.tile([P, CHUNK], f32, tag="e", name=f"e{s}_{m}_{b}")
                            if rho == 1.0:
                                nc.vector.tensor_tensor(e_t[:], t_t[:], Yt[m][:, bs], Alu.add)
                            else:
                                nc.vector.scalar_tensor_tensor(
                                    e_t[:], t_t[:], rho, Yt[m][:, bs], Alu.mult, Alu.add
                                )
                            nc.vector.scalar_tensor_tensor(
                                rnew[m][:, bs], ut[m][:, bs], -2.0 * rho, e_t[:], Alu.mult, Alu.add
                            )

    nc.compile()
    return nc


def _pack_weights(M2, M3):
    W = np.transpose(M2 * M3[:, :, None], (0, 2, 1)).astype(np.float32)  # [S,N,N] lhsT
    Wp = (
        W.reshape(S, KT // KPACK, KPACK, P, N)
        .transpose(0, 1, 3, 2, 4)
        .reshape(S, KT // KPACK, P, KPACK * N)
    )
    return np.ascontiguousarray(Wp)


def _numpy_fallback(y, M2, M3, alpha, lamb, rho):
    yT = y.T.astype(np.float32)
    z = np.zeros_like(yT)
    u = np.zeros_like(yT)
    for s in range(M2.shape[0]):
        x = M3[s][:, None] * (M2[s] @ (yT + rho * (z - u)))
        x1 = alpha[s] * x + (1.0 - alpha[s]) * z
        v = x1 + u
        t = lamb[s] / rho
        z = np.sign(v) * np.maximum(np.abs(v) - t, 0.0)
        u = v - z
    return np.ascontiguousarray(z.T)


def kernel(y, M2, M3, alpha, lamb, rho):
    global LAST_RESULT
    y = np.asarray(y, dtype=np.float32)
    M2 = np.asarray(M2, dtype=np.float32)
    M3 = np.asarray(M3, dtype=np.float32)
    alpha = np.asarray(alpha, dtype=np.float32)
    lamb = np.asarray(lamb, dtype=np.float32)
    rho_f = float(np.asarray(rho))

    if not np.all(alpha == 1.0):
        return _numpy_fallback(y, M2, M3, alpha, lamb, rho_f)

    from concourse.bass_utils import run_bass_kernel_spmd

    lam = tuple(float(l) / rho_f for l in lamb)
    key = (lam, rho_f)
    nc = _compiled.get(key)
    if nc is None:
        nc = _build(lam, rho_f)
        _compiled[key] = nc

    Wp = _pack_weights(M2, M3)
    in_maps = []
    for c in range(NCORES):
        yt_c = np.ascontiguousarray(y[c * BC : (c + 1) * BC, :].T)
        in_maps.append({"yt": yt_c, "w": Wp})

    try:
        import antenv.axon_hooks  # noqa: F401
        trace = bool(os.environ.get("BASS_TRACE"))
    except ImportError:
        # No NTFF hook registry in this image: make sure bass_utils never
        # takes the trace path (it would crash importing antenv.axon_hooks).
        os.environ["BASS_NEVER_TRACE"] = "1"
        trace = False
    res = run_bass_kernel_spmd(nc, in_maps, core_ids=list(range(NCORES)), trace=trace)
    LAST_RESULT = res

    out = np.empty((B, N), dtype=np.float32)
    for c in range(NCORES):
        out[c * BC : (c + 1) * BC, :] = res.results[c]["out"].T
    return out


# revision 12
# speedup vs baseline: 1.0550x; 1.0550x over previous
"""ADMM-Net (12 unrolled stages) on 8 TRN2 NeuronCores.

Data-parallel over the batch: each core owns a [N, B/8] column block of the
transposed problem. Per stage s (with state u, r where r = yT + rho*(z-u)):

    psum = W_s.T @ r            W_s = (diag(M3_s) M2_s).T, pre-folded on host
    t    = psum + u             (alpha == 1 fast path)
    u'   = clip(t, +-lam/rho)   == t - softthresh(t) ; becomes next u
    r'   = yT + rho*(t - 2u')   next stage matmul rhs
    z    = t - u'               (emitted only at the last stage)

Matmuls run in float32r (FP22 truncated reads, fp32 accumulate): full
1-cycle/row PE speed at 512-wide moving tiles, ~7e-4 final rel err.
All state stays fp32 in SBUF for the whole 12-stage chain; only the
per-stage weights stream from HBM.
"""

import os
import numpy as np

S, B, N = 12, 8192, 1024
NCORES = 8
BC = B // NCORES          # batch columns per core
P = 128                   # partitions
KT = N // P               # 8 contraction tiles
MT = N // P               # 8 output-row tiles
CHUNK = 512               # psum bank width (f32)
NB = BC // CHUNK          # 2 column chunks per core
KPACK = 1                 # k-slabs packed per weight DMA
E_OFFLOAD = "gpsimd"          # engine for the e = rho*t + Y op

_compiled: dict = {}
LAST_RESULT = None        # BassKernelResults of the most recent run (for test.py)


def _build(lam: tuple, rho: float):
    import concourse.tile as tile
    from concourse import bacc, mybir

    f32 = mybir.dt.float32
    f32r = mybir.dt.float32r
    Alu = mybir.AluOpType
    Act = mybir.ActivationFunctionType

    nc = bacc.Bacc("TRN2", target_bir_lowering=False, debug=False, num_devices=NCORES)
    yt_d = nc.dram_tensor("yt", [N, BC], f32r, kind="ExternalInput").ap()
    w_d = nc.dram_tensor("w", [S, KT // KPACK, P, KPACK * N], f32r, kind="ExternalInput").ap()
    out_d = nc.dram_tensor("out", [N, BC], f32, kind="ExternalOutput").ap()

    with tile.TileContext(nc) as tc:
        with (
            tc.tile_pool(name="state", bufs=1) as state,
            tc.tile_pool(name="wpool", bufs=12) as wpool,
            tc.tile_pool(name="scratch", bufs=4) as scratch,
            tc.tile_pool(name="fin", bufs=2) as fin,
            tc.tile_pool(name="psum", bufs=6, space="PSUM") as psum_pool,
        ):
            Yt = [state.tile([P, BC], f32r, tag=f"Y{i}", name=f"Y{i}") for i in range(KT)]
            for i in range(KT):
                nc.sync.dma_start(Yt[i][:, 0:CHUNK], yt_d[i * P : (i + 1) * P, 0:CHUNK])
            ut = [state.tile([P, BC], f32, tag=f"u{i}", name=f"u{i}") for i in range(MT)]
            rt = [
                [state.tile([P, BC], f32r, tag=f"r{p}_{i}", name=f"r{p}_{i}") for i in range(KT)]
                for p in range(2)
            ]

            for s in range(S):
                lam_s = float(lam[s])
                rhs = Yt if s == 0 else rt[(s - 1) % 2]
                rnew = rt[s % 2]

                slabs = [wpool.tile([P, KPACK * N], f32r, tag="w", name=f"w{s}_{i}") for i in range(KT // KPACK)]
                for kk in range(KT // KPACK):
                    nc.sync.dma_start(slabs[kk][:], w_d[s, kk, :, :])
                if s == 0:
                    # yT b1-halves: needed only mid-stage-0; issue after stage-0 weights
                    for i in range(KT):
                        nc.sync.dma_start(
                            Yt[i][:, CHUNK:BC], yt_d[i * P : (i + 1) * P, CHUNK:BC]
                        )

                def wtile(k, m):
                    sl = slabs[k // KPACK]
                    off = (k % KPACK) * N + m * P
                    return sl[:, off : off + P]

                for b in range(NB):
                    bs = slice(b * CHUNK, (b + 1) * CHUNK)
                    for m in range(MT):
                        ps = psum_pool.tile([P, CHUNK], f32, tag="ps", name=f"ps{s}_{m}_{b}")
                        for k in range(KT):
                            nc.tensor.matmul(
                                ps[:],
                                wtile(k, m),
                                rhs[k][:, bs],
                                start=(k == 0),
                                stop=(k == KT - 1),
                            )
                        t_t = scratch.tile([P, CHUNK], f32, tag="t", name=f"t{s}_{m}_{b}")
                        if s == 0:
                            # u == 0: t = psum
                            nc.scalar.activation(t_t[:], ps[:], Act.Copy)
                        else:
                            nc.vector.tensor_tensor(t_t[:], ps[:], ut[m][:, bs], Alu.add)
                        if s == S - 1:
                            uc = fin.tile([P, CHUNK], f32, tag="uc", name=f"uc{m}_{b}")
                            nc.gpsimd.tensor_scalar(uc[:], t_t[:], lam_s, -lam_s, Alu.min, Alu.max)
                            z_t = fin.tile([P, CHUNK], f32, tag="z", name=f"z{m}_{b}")
                            nc.vector.tensor_tensor(z_t[:], t_t[:], uc[:], Alu.subtract)
                            nc.sync.dma_start(out_d[m * P : (m + 1) * P, bs], z_t[:])
                        else:
                            nc.gpsimd.tensor_scalar(ut[m][:, bs], t_t[:], lam_s, -lam_s, Alu.min, Alu.max)
                            e_t = scratch.tile([P, CHUNK], f32, tag="e", name=f"e{s}_{m}_{b}")
                            if rho == 1.0:
                                nc.vector.tensor_tensor(e_t[:], t_t[:], Yt[m][:, bs], Alu.add)
                            else:
                                nc.vector.scalar_tensor_tensor(
                                    e_t[:], t_t[:], rho, Yt[m][:, bs], Alu.mult, Alu.add
                                )
                            nc.vector.scalar_tensor_tensor(
                                rnew[m][:, bs], ut[m][:, bs], -2.0 * rho, e_t[:], Alu.mult, Alu.add
                            )

    nc.compile()
    return nc


def _pack_weights(M2, M3):
    W = np.transpose(M2 * M3[:, :, None], (0, 2, 1)).astype(np.float32)  # [S,N,N] lhsT
    Wp = (
        W.reshape(S, KT // KPACK, KPACK, P, N)
        .transpose(0, 1, 3, 2, 4)
        .reshape(S, KT // KPACK, P, KPACK * N)
    )
    return np.ascontiguousarray(Wp)


def _numpy_fallback(y, M2, M3, alpha, lamb, rho):
    yT = y.T.astype(np.float32)
    z = np.zeros_like(yT)
    u = np.zeros_like(yT)
    for s in range(M2.shape[0]):
        x = M3[s][:, None] * (M2[s] @ (yT + rho * (z - u)))
        x1 = alpha[s] * x + (1.0 - alpha[s]) * z
        v = x1 + u
        t = lamb[s] / rho
        z = np.sign(v) * np.maximum(np.abs(v) - t, 0.0)
        u = v - z
    return np.ascontiguousarray(z.T)


def kernel(y, M2, M3, alpha, lamb, rho):
    global LAST_RESULT
    y = np.asarray(y, dtype=np.float32)
    M2 = np.asarray(M2, dtype=np.float32)
    M3 = np.asarray(M3, dtype=np.float32)
    alpha = np.asarray(alpha, dtype=np.float32)
    lamb = np.asarray(lamb, dtype=np.float32)
    rho_f = float(np.asarray(rho))

    if not np.all(alpha == 1.0):
        return _numpy_fallback(y, M2, M3, alpha, lamb, rho_f)

    from concourse.bass_utils import run_bass_kernel_spmd

    lam = tuple(float(l) / rho_f for l in lamb)
    key = (lam, rho_f)
    nc = _compiled.get(key)
    if nc is None:
        nc = _build(lam, rho_f)
        _compiled[key] = nc

    Wp = _pack_weights(M2, M3)
    in_maps = []
    for c in range(NCORES):
        yt_c = np.ascontiguousarray(y[c * BC : (c + 1) * BC, :].T)
        in_maps.append({"yt": yt_c, "w": Wp})

    try:
        import antenv.axon_hooks  # noqa: F401
        trace = bool(os.environ.get("BASS_TRACE"))
    except ImportError:
        # No NTFF hook registry in this image: make sure bass_utils never
        # takes the trace path (it would crash importing antenv.axon_hooks).
        os.environ["BASS_NEVER_TRACE"] = "1"
        trace = False
    res = run_bass_kernel_spmd(nc, in_maps, core_ids=list(range(NCORES)), trace=trace)
    LAST_RESULT = res

    out = np.empty((B, N), dtype=np.float32)
    for c in range(NCORES):
        out[c * BC : (c + 1) * BC, :] = res.results[c]["out"].T
    return out
